# revision 3
# baseline (speedup 1.0000x reference)
"""Trainium2 Bass kernel for nn_ChargePredict (segment_reduce).

Sharding: data-parallel over atoms with molecule-aligned shard boundaries so
segment sums stay core-local. Each core processes a fixed-size overlapping
window of NCAP atoms; one-hot indicator columns are zeroed outside the core's
own molecule range and the host discards overlap rows on gather.

Per-core pipeline (atoms on partitions, 128 per block, G blocks per tile):
  DMA X tile [128, G, 2304] (contiguous 9216B per partition row)
  DVE: I3 = sum_diag X, cross = x01*x10 + x02*x20 + x12*x21,
       X <- X^2 in place, R1 = sum_k X2, dsq = sum_diag X2
  feat: I = I3/3, nA = R1/2 - (dsq/2 + cross), nS = R1/2 + (dsq/2+cross) - I3^2/3
  LN in place via bn_stats + fused (feat-mean)*rstd
  PE: transpose ln chunks; mm1 h1T[j,at] = W1f^T lnT (6-chunk accum); ACT Silu
  PE: mm2 out2T[32,at] = W2^T h1T; +b2; transpose to [at,32]; Square f rows
  PE: segment matmul with preloaded bf16 one-hot accumulates [128 mols, 32]
  post: recip(F_u+eps); gather matmul per block; batched qeq epilogue
"""

import numpy as np
from contextlib import ExitStack

N_ATOMS = 131072
HID = 256
QD = 16
N_MOL = 1024
LN_EPS = 1e-5
QEQ_EPS = 1e-6

NCORES = 8
MPC = N_MOL // NCORES          # 128 molecules per core
NCAP = 17408                   # per-core padded atom window (136 * 128)
NB = NCAP // 128               # 136 atom blocks
G = 2                          # blocks per tile
NT = NB // G                   # 68 tiles
F3 = 3 * HID                   # 768


def _legalize_waits(nc):
    """Walrus codegen accepts at most 1 embedded sync wait per compute
    instruction (2 for DMA). Tile occasionally emits more; split the excess
    onto same-engine ENGINE_NOPs inserted immediately before the offender
    (safe: no reordering, the nop blocks the engine exactly where the wait
    previously lived)."""
    import bass_rust
    eng = {"DVE": nc.vector, "Activation": nc.scalar, "PE": nc.tensor,
           "Pool": nc.gpsimd, "SP": nc.sync}
    f = nc.m.functions[0]
    for blk in f.blocks:
        il = blk.instructions
        idx = 0
        while idx < len(il):
            ins = il[idx]
            cls = ins.__class__.__name__
            si = ins.sync_info
            if cls == "InstEventSemaphore" or not si or not si.on_wait:
                idx += 1
                continue
            limit = 1
            waits = list(si.on_wait)
            if len(waits) <= limit:
                idx += 1
                continue
            engine_name = str(getattr(ins, "engine", "")).split(".")[-1]
            e = eng.get(engine_name, nc.vector)
            excess = waits[:-limit]
            keep = waits[-limit:]
            upd = list(si.on_update) if si.on_update else []
            ins.sync_info = bass_rust.SyncInfo(on_wait=keep, on_update=upd)
            for w in excess:
                nop = e.nop(nofuse=True)
                mi = nop.ins
                for b2 in f.blocks:
                    l2 = b2.instructions
                    for k in range(len(l2) - 1, -1, -1):
                        if l2[k] is mi:
                            del l2[k]
                mi.sync_info = bass_rust.SyncInfo(on_wait=[w], on_update=[])
                il.insert(idx, mi)
                idx += 1
            idx += 1


def _validate_waits(nc):
    f = nc.m.functions[0]
    bad = []
    for blk in f.blocks:
        for ins in blk.instructions:
            if ins.__class__.__name__ == 'InstEventSemaphore':
                continue
            n = (len(ins.sync_info.on_wait)
                 if ins.sync_info and ins.sync_info.on_wait else 0)
            if n > 1:
                bad.append((ins.name, ins.__class__.__name__, n))
    return bad


def _build_program(variant=0):
    import concourse.bass as bass
    import concourse.tile as tile
    from concourse import mybir

    f32 = mybir.dt.float32
    bf16 = mybir.dt.bfloat16
    AF = mybir.ActivationFunctionType
    OP = mybir.AluOpType
    AX = mybir.AxisListType

    nc = bass.Bass("TRN2", target_bir_lowering=False, debug=False,
                   num_devices=NCORES)

    x_d = nc.dram_tensor("x", [NCAP, 2304], f32, kind="ExternalInput").ap()
    qv_d = nc.dram_tensor("qv", [128, NB], bf16, kind="ExternalInput").ap()
    ohn_d = nc.dram_tensor("ohn", [NCAP, 128], bf16, kind="ExternalInput").ap()
    oht_d = nc.dram_tensor("oht", [128, NCAP], bf16, kind="ExternalInput").ap()
    w1_d = nc.dram_tensor("w1", [F3, 256], f32, kind="ExternalInput").ap()
    b1_d = nc.dram_tensor("b1", [2, 128], f32, kind="ExternalInput").ap()
    w2_d = nc.dram_tensor("w2", [256, 32], f32, kind="ExternalInput").ap()
    b2_d = nc.dram_tensor("b2", [32, 1], f32, kind="ExternalInput").ap()
    id_d = nc.dram_tensor("ident", [128, 128], f32, kind="ExternalInput").ap()
    out_d = nc.dram_tensor("out", [NCAP, QD], f32, kind="ExternalOutput").ap()

    with tile.TileContext(nc) as tc, ExitStack() as ctx:
        singles = ctx.enter_context(tc.tile_pool(name="singles", bufs=1))
        xp = ctx.enter_context(tc.tile_pool(name="xp", bufs=1))
        fp = ctx.enter_context(tc.tile_pool(name="fp", bufs=1))
        sm = ctx.enter_context(tc.tile_pool(name="sm", bufs=2))
        lt = ctx.enter_context(tc.tile_pool(name="lt", bufs=2))
        ps = ctx.enter_context(tc.tile_pool(name="ps", bufs=2, space="PSUM"))
        ps_t = ctx.enter_context(tc.tile_pool(name="ps_t", bufs=3, space="PSUM"))
        ps_seg = ctx.enter_context(tc.tile_pool(name="ps_seg", bufs=1, space="PSUM"))
        big = ctx.enter_context(tc.tile_pool(name="big", bufs=1))

        # ---- constants / weights / one-hots (loaded once) ----
        ident = singles.tile([128, 128], f32)
        nc.sync.dma_start(out=ident, in_=id_d)
        w1_sb = singles.tile([128, 6, 256], f32)
        nc.sync.dma_start(out=w1_sb, in_=w1_d.rearrange("(c p) j -> p c j", p=128))
        b1_sb = singles.tile([128, 2], f32)
        nc.sync.dma_start(out=b1_sb, in_=b1_d.rearrange("c p -> p c"))
        w2_sb = singles.tile([128, 2, 32], f32)
        nc.sync.dma_start(out=w2_sb, in_=w2_d.rearrange("(c p) q -> p c q", p=128))
        b2_sb = singles.tile([32, 1], f32)
        nc.sync.dma_start(out=b2_sb, in_=b2_d)
        qv_sb = singles.tile([128, NB], bf16)
        nc.sync.dma_start(out=qv_sb, in_=qv_d)
        eps_sb = singles.tile([128, 1], f32)
        nc.vector.memset(eps_sb, LN_EPS)
        dmy = singles.tile([1, 8], bf16)
        nc.vector.memset(dmy, 0.0)
        nc._legalize_dummy = dmy
        ohn_all = singles.tile([128, NB, 128], bf16)
        nc.sync.dma_start(out=ohn_all,
                          in_=ohn_d.rearrange("(b p) m -> p b m", p=128))
        oht_all = singles.tile([128, NB, 128], bf16)
        nc.sync.dma_start(out=oht_all,
                          in_=oht_d.rearrange("p (b a) -> p b a", a=128))

        # persistent staging across tiles
        cf_st = big.tile([128, NB, 32], bf16)     # [charges | f_u] atom-major
        chg_st = big.tile([128, NB, QD], f32)     # fp32 charges (output base)
        gath = big.tile([128, NB, 32], bf16)      # gathered [Q_u | recip]
        seg_ps = ps_seg.tile([128, 32], f32)      # [Q_u | F_u] per-mol accum

        for t in range(NT):
            xt = xp.tile([128, G, 2304], f32, tag=f"xt{t % 2}")
            a0 = t * G * 128
            nc.gpsimd.dma_start(
                out=xt,
                in_=x_d[a0:a0 + G * 128].rearrange("(g p) e -> p g e", p=128))
            x4 = xt.rearrange("p g (h k) -> p g h k", k=9)

            # reads of raw X first (TT ops; order rotated by variant to
            # dodge rare 3-wait schedules -- walrus allows max 2 per inst)
            scratch = fp.tile([128, G, F3], f32, tag="scratch")
            stg = scratch.rearrange("p g (h c) -> p g h c", c=3)
            i3 = fp.tile([128, G, 256], f32, tag="i3")
            first_ops = [
                lambda: nc.vector.tensor_mul(stg[:, :, :, 0:2],
                                             x4[:, :, :, 1:3],
                                             x4[:, :, :, 3:7:3]),
                lambda: nc.vector.tensor_mul(stg[:, :, :, 2:3],
                                             x4[:, :, :, 5:6],
                                             x4[:, :, :, 7:8]),
                lambda: nc.vector.tensor_add(i3, x4[:, :, :, 0],
                                             x4[:, :, :, 4]),
            ]
            for k in range(3):
                first_ops[(k + variant) % 3]()
            nc.vector.tensor_add(i3, i3, x4[:, :, :, 8])
            crs = fp.tile([128, G, 256], f32, tag="crs")
            nc.vector.reduce_sum(crs, stg, axis=AX.X)
            isq = fp.tile([128, G, 256], f32, tag="isq")
            nc.scalar.activation(isq, i3, AF.Square, scale=0.57735026919)

            # square X in place, then reduce
            nc.vector.tensor_mul(xt, xt, xt)
            r1 = fp.tile([128, G, 256], f32, tag="r1")
            nc.vector.reduce_sum(r1, x4, axis=AX.X)
            dsq = fp.tile([128, G, 256], f32, tag="dsq")
            nc.vector.reduce_sum(dsq, x4[:, :, :, 0:9:4], axis=AX.X)

            # feat assembled into scratch (stg no longer needed)
            feat = scratch
            half = fp.tile([128, G, 256], f32, tag="half")
            nc.vector.scalar_tensor_tensor(half, dsq, 0.5, crs, OP.mult, OP.add)
            nc.vector.scalar_tensor_tensor(feat[:, :, 256:512], r1, 0.5, half,
                                           OP.mult, OP.subtract)
            nc.vector.scalar_tensor_tensor(half, r1, 0.5, half, OP.mult, OP.add)
            nc.vector.tensor_sub(feat[:, :, 512:768], half, isq)
            nc.vector.tensor_scalar_mul(feat[:, :, 0:256], i3, 1.0 / 3.0)

            # ---- LayerNorm (in place on feat) ----
            mv = sm.tile([128, G, 2], f32, tag="mv")
            for g in range(G):
                stats = sm.tile([128, 3, 6], f32, tag="stats")
                fr = feat[:, g, :].rearrange("p (s d) -> p s d", s=3)
                for s in range(3):
                    nc.vector.bn_stats(out=stats[:, s, :], in_=fr[:, s, :])
                nc.vector.bn_aggr(out=mv[:, g, :], in_=stats)
            rstd = sm.tile([128, G], f32, tag="rstd")
            nc.scalar.activation(rstd, mv[:, :, 1], AF.Sqrt, bias=eps_sb)
            nc.vector.reciprocal(rstd, rstd)
            for g in range(G):
                nc.vector.tensor_scalar(feat[:, g, :], feat[:, g, :],
                                        mv[:, g, 0:1], rstd[:, g:g + 1],
                                        OP.subtract, OP.mult)

            # ---- transpose ln -> lnT chunks [128f, G*128at] ----
            lnT = lt.tile([128, 6, G, 128], f32, tag="lnT")
            for c in range(6):
                for g in range(G):
                    tp = ps_t.tile([128, 128], f32, tag="tp")
                    nc.tensor.transpose(tp, feat[:, g, 128 * c:128 * (c + 1)],
                                        ident)
                    nc.scalar.activation(lnT[:, c, g, :], tp, AF.Copy)

            # ---- mm1 + Silu ----
            h1T = lt.tile([128, 2, G, 128], f32, tag="h1T")
            for jb in range(2):
                o1 = ps.tile([128, G * 128], f32, tag="mm")
                for c in range(6):
                    nc.tensor.matmul(o1, w1_sb[:, c, 128 * jb:128 * (jb + 1)],
                                     lnT[:, c, :, :].rearrange("p g a -> p (g a)"),
                                     start=(c == 0), stop=(c == 5))
                nc.scalar.activation(
                    h1T[:, jb, :, :].rearrange("p g a -> p (g a)"), o1,
                    AF.Silu, bias=b1_sb[:, jb:jb + 1])

            # ---- mm2 ----
            o2 = ps.tile([32, G * 128], f32, tag="mm")
            for c2 in range(2):
                nc.tensor.matmul(o2, w2_sb[:, c2, :],
                                 h1T[:, c2, :, :].rearrange("p g a -> p (g a)"),
                                 start=(c2 == 0), stop=(c2 == 1))
            o2c = sm.tile([32, G * 128], f32, tag="o2c")
            nc.vector.tensor_scalar_add(o2c, o2, b2_sb)

            # ---- atom-major + f_u square + segment accumulate ----
            for g in range(G):
                tp2 = ps_t.tile([128, 32], f32, tag="tp")
                nc.tensor.transpose(tp2, o2c[:, 128 * g:128 * (g + 1)],
                                    ident[0:32, 0:32])
                b = t * G + g
                nc.scalar.activation(chg_st[:, b, :], tp2[:, 0:16], AF.Copy)
                nc.scalar.activation(cf_st[:, b, 0:16], tp2[:, 0:16], AF.Copy)
                nc.scalar.activation(cf_st[:, b, 16:32], tp2[:, 16:32],
                                     AF.Square)
                nc.tensor.matmul(seg_ps, ohn_all[:, b, :], cf_st[:, b, :],
                                 start=(b == 0), stop=(b == NB - 1))

        # ---- molecule-level post ----
        mtmp = singles.tile([128, 16], f32)
        nc.vector.tensor_scalar_add(mtmp, seg_ps[:, 16:32], QEQ_EPS)
        nc.vector.reciprocal(mtmp, mtmp)
        mvals = singles.tile([128, 32], bf16)
        nc.vector.tensor_copy(mvals[:, 16:32], mtmp)
        nc.vector.tensor_copy(mvals[:, 0:16], seg_ps[:, 0:16])

        for b in range(NB):
            gp = ps_t.tile([128, 32], f32, tag="tp")
            nc.tensor.matmul(gp, oht_all[:, b, :], mvals, start=True, stop=True)
            nc.scalar.activation(gath[:, b, :], gp, AF.Copy)

        # ---- batched qeq epilogue ----
        qbc = bass.AP(tensor=qv_sb.tensor, offset=qv_sb.offset,
                      ap=[qv_sb.ap[0], [qv_sb.ap[1][0], NB], [0, QD]])
        # dq = Q - Q_u  (in place over gath Qu slot)
        nc.vector.tensor_tensor(gath[:, :, 0:16], qbc, gath[:, :, 0:16],
                                OP.subtract)
        # scale = f_u * recip (in place over gath recip slot)
        nc.vector.tensor_mul(gath[:, :, 16:32], cf_st[:, :, 16:32],
                             gath[:, :, 16:32])
        corr = xp.tile([128, NB, QD], f32, tag="xt0")
        nc.vector.tensor_mul(corr, gath[:, :, 0:16], gath[:, :, 16:32])
        res_t = xp.tile([128, NB, QD], f32, tag="xt1")
        nc.vector.tensor_add(res_t, chg_st, corr)
        nc.sync.dma_start(
            out=out_d.rearrange("(b p) q -> p b q", p=128), in_=res_t)

    return nc


LAST_EXEC_NS = None
LAST_RESULTS = None


def kernel(X, Q, ln_w, ln_b, W1, b1, W2, b2, batch):
    import ml_dtypes
    from concourse.bass_utils import run_bass_kernel_spmd

    bf = ml_dtypes.bfloat16
    X = np.ascontiguousarray(np.asarray(X, dtype=np.float32)).reshape(N_ATOMS, 2304)
    Q = np.asarray(Q, dtype=np.float32)
    batch = np.asarray(batch, dtype=np.int64)

    edges = np.searchsorted(batch, np.arange(0, N_MOL + 1, MPC))
    edges[0] = 0
    edges[-1] = N_ATOMS

    W1f = (np.asarray(ln_w, np.float32)[:, None] * np.asarray(W1, np.float32))
    b1f = (np.asarray(b1, np.float32)
           + np.asarray(ln_b, np.float32) @ np.asarray(W1, np.float32))
    W2 = np.asarray(W2, np.float32)
    b2v = np.asarray(b2, np.float32)

    in_maps = []
    starts = []
    for c in range(NCORES):
        s, e = int(edges[c]), int(edges[c + 1])
        assert e - s <= NCAP, f"core {c} needs {e - s} > NCAP"
        start = min(s, N_ATOMS - NCAP)
        starts.append(start)
        bc = batch[start:start + NCAP]
        rel = (bc - c * MPC).astype(np.int64)
        idx = np.arange(NCAP) + start
        valid = (idx >= s) & (idx < e) & (rel >= 0) & (rel < MPC)
        ohn = np.zeros((NCAP, 128), dtype=np.float32)
        rows = np.nonzero(valid)[0]
        ohn[rows, rel[valid]] = 1.0
        qv = Q[start:start + NCAP].reshape(NB, 128).T
        in_maps.append({
            "x": X[start:start + NCAP],
            "qv": np.ascontiguousarray(qv.astype(bf)),
            "ohn": ohn.astype(bf),
            "oht": np.ascontiguousarray(ohn.T.astype(bf)),
            "w1": W1f,
            "b1": np.ascontiguousarray(b1f.reshape(2, 128)),
            "w2": W2,
            "b2": b2v.reshape(32, 1),
            "ident": np.eye(128, dtype=np.float32),
        })

    global LAST_EXEC_NS
    nc = None
    for v in range(4):
        cand = _build_program(variant=v)
        _legalize_waits(cand)
        bad = _validate_waits(cand)
        if not bad:
            nc = cand
            break
        print(f"kernel build variant {v} still has over-limit waits: {bad[:3]}")
    assert nc is not None, "no clean build variant found"
    res = run_bass_kernel_spmd(nc, in_maps, core_ids=list(range(NCORES)))
    global LAST_RESULTS
    LAST_RESULTS = res
    LAST_EXEC_NS = res.exec_time_ns

    out = np.empty((N_ATOMS, QD), dtype=np.float32)
    for c in range(NCORES):
        s, e = int(edges[c]), int(edges[c + 1])
        r = res.results[c]["out"]
        out[s:e] = r[s - starts[c]:e - starts[c]]
    return out



# revision 16
# speedup vs baseline: 1.8051x; 1.8051x over previous
"""Trainium2 Bass kernel for nn_ChargePredict (segment_reduce), v2.

Sharding: data-parallel over atoms with molecule-aligned shard boundaries so
segment sums stay core-local. Each core processes a fixed-size overlapping
window of NCAP atoms; one-hot indicator columns are zeroed outside the core's
own molecule range and the host discards overlap rows on gather.

v2 strategy (baseline was DVE-bound at ~2.16 ms):
  - X is downconverted to bf16 on the host: halves HBM traffic and enables
    the DVE 2x 2-byte mode on packed elementwise ops.
  - squares+group-sums are computed as add-trees (2x packed adds) instead of
    1-elem/cycle tensor_reduce.
  - work is spread across DVE / ACT (scalar) / Pool (gpsimd) so no engine
    holds more than ~9 us per 256-atom tile.
  - ln/out transposes run on the DMA XBAR (dma_start_transpose, 2-byte)
    instead of PE transposes + PSUM->SBUF copies.
  - all matmuls in bf16 (1 cycle/row vs 4 for fp32), b2 folded in via an
    augmented K=1 matmul row.

Per-core pipeline (atoms on partitions, 128 per block, G=2 blocks per tile):
  DMA xt [128, G, 2304] bf16
  DVE: cross-pair products, s-tree (xsq pair sums -> r1, dsq), qeq chain
  ACT: xsq = x^2 (fp16), isq, I=i3/3, Silu, |f|^2, o2 copy
  Pool: i3 adds, cross tree, LN apply (tensor_scalar)
  XBAR: ln -> lnT blocked transpose; o2 -> atom-major cf_st
  PE: mm1 (6-chunk accum), mm2 (+b2 aug row), segment one-hot matmul
  post: recip(F_u+eps), gather matmuls (streamed oht), batched qeq epilogue
"""

import numpy as np
from contextlib import ExitStack

N_ATOMS = 131072
HID = 256
QD = 16
N_MOL = 1024
LN_EPS = 1e-5
QEQ_EPS = 1e-6

NCORES = 8
MPC = N_MOL // NCORES          # 128 molecules per core
NCAP = 17408                   # per-core padded atom window (136 * 128)
NB = NCAP // 128               # 136 atom blocks
G = 4                          # blocks per tile
NT = NB // G                   # 34 tiles
F3 = 3 * HID                   # 768


def _legalize_waits(nc):
    """Walrus codegen accepts at most 1 embedded sync wait per compute
    instruction (2 for DMA). Tile occasionally emits more; split the excess
    onto same-engine ENGINE_NOPs inserted immediately before the offender
    (safe: no reordering, the nop blocks the engine exactly where the wait
    previously lived)."""
    import bass_rust
    eng = {"DVE": nc.vector, "Activation": nc.scalar, "PE": nc.tensor,
           "Pool": nc.gpsimd, "SP": nc.sync}
    f = nc.m.functions[0]
    for blk in f.blocks:
        il = blk.instructions
        idx = 0
        while idx < len(il):
            ins = il[idx]
            cls = ins.__class__.__name__
            si = ins.sync_info
            if cls == "InstEventSemaphore" or not si or not si.on_wait:
                idx += 1
                continue
            limit = 1
            waits = list(si.on_wait)
            if len(waits) <= limit:
                idx += 1
                continue
            engine_name = str(getattr(ins, "engine", "")).split(".")[-1]
            e = eng.get(engine_name, nc.vector)
            excess = waits[:-limit]
            keep = waits[-limit:]
            upd = list(si.on_update) if si.on_update else []
            ins.sync_info = bass_rust.SyncInfo(on_wait=keep, on_update=upd)
            for w in excess:
                nop = e.nop(nofuse=True)
                mi = nop.ins
                for b2 in f.blocks:
                    l2 = b2.instructions
                    for k in range(len(l2) - 1, -1, -1):
                        if l2[k] is mi:
                            del l2[k]
                mi.sync_info = bass_rust.SyncInfo(on_wait=[w], on_update=[])
                il.insert(idx, mi)
                idx += 1
            idx += 1


def _validate_waits(nc):
    f = nc.m.functions[0]
    bad = []
    for blk in f.blocks:
        for ins in blk.instructions:
            if ins.__class__.__name__ == 'InstEventSemaphore':
                continue
            n = (len(ins.sync_info.on_wait)
                 if ins.sync_info and ins.sync_info.on_wait else 0)
            if n > 1:
                bad.append((ins.name, ins.__class__.__name__, n))
    return bad


def _build_program(variant=0):
    import concourse.bass as bass
    import concourse.tile as tile
    from concourse import mybir

    f32 = mybir.dt.float32
    bf16 = mybir.dt.bfloat16
    fp16 = mybir.dt.float16
    AF = mybir.ActivationFunctionType
    OP = mybir.AluOpType

    nc = bass.Bass("TRN2", target_bir_lowering=False, debug=False,
                   num_devices=NCORES)

    x_d = nc.dram_tensor("x", [NCAP, 2304], bf16, kind="ExternalInput").ap()
    qv_d = nc.dram_tensor("qv", [128, NB], bf16, kind="ExternalInput").ap()
    ohn_d = nc.dram_tensor("ohn", [NCAP, 128], bf16, kind="ExternalInput").ap()
    oht_d = nc.dram_tensor("oht", [128, NCAP], bf16, kind="ExternalInput").ap()
    w1_d = nc.dram_tensor("w1", [F3, 256], bf16, kind="ExternalInput").ap()
    b1_d = nc.dram_tensor("b1", [2, 128], f32, kind="ExternalInput").ap()
    w2_d = nc.dram_tensor("w2", [256, 32], bf16, kind="ExternalInput").ap()
    b2_d = nc.dram_tensor("b2", [1, 32], bf16, kind="ExternalInput").ap()
    out_d = nc.dram_tensor("out", [NCAP, QD], f32, kind="ExternalOutput").ap()

    with tile.TileContext(nc) as tc, ExitStack() as ctx:
        singles = ctx.enter_context(tc.tile_pool(name="singles", bufs=1))
        xp = ctx.enter_context(tc.tile_pool(name="xp", bufs=2))
        xq = ctx.enter_context(tc.tile_pool(name="xq", bufs=1))
        fp = ctx.enter_context(tc.tile_pool(name="fp", bufs=2))
        sm = ctx.enter_context(tc.tile_pool(name="sm", bufs=2))
        ps = ctx.enter_context(tc.tile_pool(name="ps", bufs=2, space="PSUM"))
        ps_seg = ctx.enter_context(tc.tile_pool(name="ps_seg", bufs=1, space="PSUM"))
        ps_g = ctx.enter_context(tc.tile_pool(name="ps_g", bufs=1, space="PSUM"))
        big = ctx.enter_context(tc.tile_pool(name="big", bufs=1))

        # ---- constants / weights / one-hots (loaded once) ----
        w1_sb = singles.tile([128, 6, 2, 128], bf16)
        nc.sync.dma_start(out=w1_sb,
                          in_=w1_d.rearrange("(c p) (jb j) -> p c jb j",
                                             p=128, j=128))
        b1_sb = singles.tile([128, 2], f32)
        nc.sync.dma_start(out=b1_sb, in_=b1_d.rearrange("c p -> p c"))
        w2_sb = singles.tile([128, 2, 32], bf16)
        nc.sync.dma_start(out=w2_sb, in_=w2_d.rearrange("(c p) q -> p c q", p=128))
        b2_sb = singles.tile([1, 32], bf16)
        nc.sync.dma_start(out=b2_sb, in_=b2_d)
        qv_sb = singles.tile([128, NB], bf16)
        nc.sync.dma_start(out=qv_sb, in_=qv_d)
        eps_sb = singles.tile([128, 1], f32)
        nc.vector.memset(eps_sb, LN_EPS)
        ones_sb = singles.tile([1, G * 128], bf16)
        nc.vector.memset(ones_sb, 1.0)
        dmy = singles.tile([1, 8], bf16)
        nc.vector.memset(dmy, 0.0)
        nc._legalize_dummy = dmy

        # persistent staging across tiles
        cf_st = big.tile([128, NB, 32], bf16)     # [charges | f_u] atom-major
        gath = big.tile([128, NB, 32], bf16)      # gathered [Q_u | recip]
        res_t = big.tile([128, NB, QD], f32)
        seg_ps = ps_seg.tile([128, 32], f32)      # [Q_u | F_u] per-mol accum

        # Software-pipelined loop: per iteration emit front_a(t), then
        # back(t-1), then front_b(t), so each engine's in-order stream
        # leads with ready work and never glues tile t's tail to tile
        # t+1's head. 2 tiles in flight (pool bufs=2).
        state = {}
        OB = 16          # ohn blocks per streamed batch

        def load_ohn(i):
            if i * OB >= NB:
                return
            nb = min(OB, NB - i * OB)
            ohnb = sm.tile([128, OB, 128], bf16, tag="ohnb")
            nc.gpsimd.dma_start(
                out=ohnb[:, 0:nb, :],
                in_=ohn_d[i * OB * 128:(i * OB + nb) * 128].rearrange(
                    "(b p) m -> p b m", p=128))
            state[("ohn", i)] = ohnb

        def front_a(t):
            a0 = t * G * 128
            xt = xp.tile([128, G, 2304], bf16, tag="xt")
            nc.sync.dma_start(
                out=xt,
                in_=x_d[a0:a0 + G * 128].rearrange("(g p) e -> p g e", p=128))
            x4 = xt.rearrange("p g (h k) -> p g h k", k=9)

            # raw-x consumers (order rotated by variant for legalize retry)
            stg = fp.tile([128, G, 256, 3], bf16, tag="stg")
            i3 = fp.tile([128, G, 256], bf16, tag="i3")
            xsq = xq.tile([128, G, 2304], fp16, tag="xsq")
            first_ops = [
                # ACT: halved squares (scale^2 = 1/2) so the s-tree yields
                # r1/2 and dsq/2 directly -> qeq chain is pure tensor_tensor
                lambda: nc.scalar.activation(xsq, xt, AF.Square,
                                             scale=0.70710678118),
                lambda: nc.vector.tensor_mul(stg[:, :, :, 0:2],
                                             x4[:, :, :, 1:3],
                                             x4[:, :, :, 3:7:3]),
                lambda: nc.gpsimd.tensor_mul(stg[:, :, :, 2],
                                             x4[:, :, :, 5], x4[:, :, :, 7]),
                lambda: nc.gpsimd.tensor_add(i3, x4[:, :, :, 0],
                                             x4[:, :, :, 4]),
            ]
            for k in range(4):
                first_ops[(k + variant) % 4]()
            nc.gpsimd.tensor_add(i3, i3, x4[:, :, :, 8])
            x4q = xsq.rearrange("p g (h k) -> p g h k", k=9)

            # cross tree early on Pool (only needs stg)
            crs = fp.tile([128, G, 256], bf16, tag="crs")
            nc.gpsimd.tensor_add(crs, stg[:, :, :, 0], stg[:, :, :, 1])
            nc.gpsimd.tensor_add(crs, crs, stg[:, :, :, 2])

            # s-tree head (DVE): pair sums of halved squares
            s1 = fp.tile([128, G, 256, 4], fp16, tag="s1")
            nc.vector.tensor_add(s1, x4q[:, :, :, 0:4], x4q[:, :, :, 4:8])
            dsq = fp.tile([128, G, 256], bf16, tag="dsq")
            nc.gpsimd.tensor_add(dsq, s1[:, :, :, 0], x4q[:, :, :, 8])
            nc.vector.tensor_add(s1[:, :, :, 0:2], s1[:, :, :, 0:2],
                                 s1[:, :, :, 2:4])
            state[("fa", t)] = (x4q, s1, i3, crs, dsq)

        def front_b(t):
            x4q, s1, i3, crs, dsq = state.pop(("fa", t))
            # s-tree tail: r1/2
            s3 = fp.tile([128, G, 256], fp16, tag="s3")
            nc.gpsimd.tensor_add(s3, s1[:, :, :, 0], s1[:, :, :, 1])
            r1 = fp.tile([128, G, 256], bf16, tag="r1")
            nc.vector.tensor_add(r1, s3, x4q[:, :, :, 8])

            # isq = i3^2/3 (ACT); I = i3/3 (DVE tensor_scalar, 4x mode)
            isq = fp.tile([128, G, 256], bf16, tag="isq")
            nc.scalar.activation(isq, i3, AF.Square, scale=0.57735026919)
            feat = fp.tile([128, G, F3], bf16, tag="feat")
            nc.vector.tensor_scalar_mul(feat[:, :, 0:256], i3, 1.0 / 3.0)

            # feat chain (DVE, all tensor_tensor 2x):
            # nA = r1/2 - (dsq/2 + crs), nS = r1/2 + (dsq/2 + crs) - isq
            half = fp.tile([128, G, 256], bf16, tag="half")
            nc.vector.tensor_add(half, dsq, crs)
            nc.vector.tensor_sub(feat[:, :, 256:512], r1, half)
            nc.vector.tensor_add(half, r1, half)
            nc.vector.tensor_sub(feat[:, :, 512:768], half, isq)

            # LayerNorm stats (DVE); sqrt/recip/apply happen in back(t)
            mv = sm.tile([128, G, 2], f32, tag="mv")
            for g in range(G):
                stats = sm.tile([128, 2, 6], f32, tag=f"stats{g}")
                fr = feat[:, g, :].rearrange("p (s d) -> p s d", s=2)
                for s in range(2):
                    nc.vector.bn_stats(out=stats[:, s, :], in_=fr[:, s, :])
                nc.vector.bn_aggr(out=mv[:, g, :], in_=stats)
            state[("fb", t)] = (feat, mv)

        def back(t):
            b0 = t * G
            feat, mv = state.pop(("fb", t))
            # rstd + LN apply (tensor_scalar gets the 4x DVE mode)
            rstd = sm.tile([128, G], f32, tag="rstd")
            nc.scalar.activation(rstd, mv[:, :, 1], AF.Sqrt, bias=eps_sb)
            nc.vector.reciprocal(rstd, rstd)
            ln = fp.tile([128, G, F3], bf16, tag="ln")
            for g in range(G):
                nc.vector.tensor_scalar(ln[:, g, :], feat[:, g, :],
                                        mv[:, g, 0:1], rstd[:, g:g + 1],
                                        OP.subtract, OP.mult)

            # XBAR transpose ln -> lnT [128f, c=(g*6+ch), 128a]
            lnT = fp.tile([128, G * 6, 128], bf16, tag="lnT")
            nc.sync.dma_start_transpose(out=lnT,
                                        in_=ln.rearrange("p g f -> p (g f)"))

            # mm1 + Silu
            h1T = fp.tile([128, 2, G * 128], bf16, tag="h1T")
            for jb in range(2):
                o1 = ps.tile([128, G * 128], f32, tag=f"mm{jb}")
                for c in range(6):
                    nc.tensor.matmul(o1, w1_sb[:, c, jb, :], lnT[:, c::6, :],
                                     start=(c == 0), stop=(c == 5))
                nc.scalar.activation(h1T[:, jb, :], o1, AF.Silu,
                                     bias=b1_sb[:, jb:jb + 1])

            # mm2 (+ b2 via K=1 augmented row)
            o2 = ps.tile([32, G * 128], f32, tag="o2")
            nc.tensor.matmul(o2, w2_sb[:, 0, :], h1T[:, 0, :],
                             start=True, stop=False)
            nc.tensor.matmul(o2, w2_sb[:, 1, :], h1T[:, 1, :],
                             start=False, stop=False)
            nc.tensor.matmul(o2, b2_sb, ones_sb, start=False, stop=True)
            o2c = sm.tile([32, G * 128], bf16, tag="o2c")
            nc.scalar.activation(o2c, o2, AF.Copy)

            # atom-major via XBAR, f_u squares in place, segment accumulate
            nc.sync.dma_start_transpose(out=cf_st[:, b0:b0 + G, :], in_=o2c)
            nc.scalar.activation(cf_st[:, b0:b0 + G, QD:32],
                                 cf_st[:, b0:b0 + G, QD:32], AF.Square)
            for g in range(G):
                b = b0 + g
                ohnb = state[("ohn", b // OB)]
                nc.tensor.matmul(seg_ps, ohnb[:, b % OB, :], cf_st[:, b, :],
                                 start=(b == 0), stop=(b == NB - 1))

        load_ohn(0)
        for t in range(NT + 1):
            # prefetch the ohn batch 2 tiles before back() first needs it
            if t % (OB // G) == (OB // G) - 2:
                load_ohn(t // (OB // G) + 1)
            if t < NT:
                front_a(t)
            if t > 0:
                back(t - 1)
            if t < NT:
                front_b(t)

        # ---- molecule-level post ----
        mtmp = singles.tile([128, QD], f32)
        nc.vector.tensor_scalar_add(mtmp, seg_ps[:, QD:32], QEQ_EPS)
        nc.vector.reciprocal(mtmp, mtmp)
        mvals = singles.tile([128, 32], bf16)
        nc.vector.tensor_copy(mvals[:, QD:32], mtmp)
        nc.vector.tensor_copy(mvals[:, 0:QD], seg_ps[:, 0:QD])

        BB = 16
        for s0 in range(0, NB, BB):
            nb = min(BB, NB - s0)
            ohtb = sm.tile([128, BB, 128], bf16, tag="ohtb")
            nc.gpsimd.dma_start(
                out=ohtb[:, 0:nb, :],
                in_=oht_d[:, s0 * 128:(s0 + nb) * 128].rearrange(
                    "p (b a) -> p b a", a=128))
            gp = ps_g.tile([128, BB, 32], f32, tag="gp")
            for i in range(nb):
                nc.tensor.matmul(gp[:, i, :], ohtb[:, i, :], mvals,
                                 start=True, stop=True)
            nc.vector.tensor_copy(gath[:, s0:s0 + nb, :], gp[:, 0:nb, :])

        # ---- batched qeq epilogue ----
        qbc = bass.AP(tensor=qv_sb.tensor, offset=qv_sb.offset,
                      ap=[qv_sb.ap[0], [qv_sb.ap[1][0], NB], [0, QD]])
        # dq = Q - Q_u  (in place over gath Qu slot)
        nc.vector.tensor_tensor(gath[:, :, 0:QD], qbc, gath[:, :, 0:QD],
                                OP.subtract)
        # scale = f_u * recip (in place over gath recip slot)
        nc.vector.tensor_mul(gath[:, :, QD:32], cf_st[:, :, QD:32],
                             gath[:, :, QD:32])
        # corr = dq * scale (in place over gath Qu slot)
        nc.vector.tensor_mul(gath[:, :, 0:QD], gath[:, :, 0:QD],
                             gath[:, :, QD:32])
        nc.vector.tensor_add(res_t, cf_st[:, :, 0:QD], gath[:, :, 0:QD])
        nc.sync.dma_start(
            out=out_d.rearrange("(b p) q -> p b q", p=128), in_=res_t)

    return nc


LAST_EXEC_NS = None
LAST_RESULTS = None


def kernel(X, Q, ln_w, ln_b, W1, b1, W2, b2, batch):
    import ml_dtypes
    from concourse.bass_utils import run_bass_kernel_spmd

    bf = ml_dtypes.bfloat16
    X = np.ascontiguousarray(np.asarray(X, dtype=np.float32)).reshape(N_ATOMS, 2304)
    Xbf = X.astype(bf)
    Q = np.asarray(Q, dtype=np.float32)
    batch = np.asarray(batch, dtype=np.int64)

    edges = np.searchsorted(batch, np.arange(0, N_MOL + 1, MPC))
    edges[0] = 0
    edges[-1] = N_ATOMS

    W1f = (np.asarray(ln_w, np.float32)[:, None] * np.asarray(W1, np.float32))
    b1f = (np.asarray(b1, np.float32)
           + np.asarray(ln_b, np.float32) @ np.asarray(W1, np.float32))
    W2f = np.asarray(W2, np.float32)
    b2v = np.asarray(b2, np.float32)

    in_maps = []
    starts = []
    for c in range(NCORES):
        s, e = int(edges[c]), int(edges[c + 1])
        assert e - s <= NCAP, f"core {c} needs {e - s} > NCAP"
        start = min(s, N_ATOMS - NCAP)
        starts.append(start)
        bc = batch[start:start + NCAP]
        rel = (bc - c * MPC).astype(np.int64)
        idx = np.arange(NCAP) + start
        valid = (idx >= s) & (idx < e) & (rel >= 0) & (rel < MPC)
        ohn = np.zeros((NCAP, 128), dtype=np.float32)
        rows = np.nonzero(valid)[0]
        ohn[rows, rel[valid]] = 1.0
        ohn = ohn.astype(bf)
        qv = Q[start:start + NCAP].reshape(NB, 128).T
        in_maps.append({
            "x": Xbf[start:start + NCAP],
            "qv": np.ascontiguousarray(qv.astype(bf)),
            "ohn": ohn,
            "oht": np.ascontiguousarray(ohn.T),
            "w1": W1f.astype(bf),
            "b1": np.ascontiguousarray(b1f.reshape(2, 128)),
            "w2": W2f.astype(bf),
            "b2": b2v.reshape(1, 32).astype(bf),
        })

    global LAST_EXEC_NS, LAST_RESULTS
    nc = None
    for v in range(4):
        cand = _build_program(variant=v)
        _legalize_waits(cand)
        bad = _validate_waits(cand)
        if not bad:
            nc = cand
            break
        print(f"kernel build variant {v} still has over-limit waits: {bad[:3]}")
    assert nc is not None, "no clean build variant found"
    res = run_bass_kernel_spmd(nc, in_maps, core_ids=list(range(NCORES)))
    LAST_RESULTS = res
    LAST_EXEC_NS = res.exec_time_ns

    out = np.empty((N_ATOMS, QD), dtype=np.float32)
    for c in range(NCORES):
        s, e = int(edges[c]), int(edges[c + 1])
        r = res.results[c]["out"]
        out[s:e] = r[s - starts[c]:e - starts[c]]
    return out
